# revision 7
# baseline (speedup 1.0000x reference)
"""BANLayer Trainium2 kernel.

Math (reference):
    v_ = relu(v @ ((v_g/||v_W||) v_W).T + v_b)          [B, hk]
    q_ = relu(q @ ((q_g/||q_W||) q_W).T + q_b)          [B, hk]
    att_maps[b,i,j] = v_[b,i] * q_[b,j]                 [B, hk, hk]
    logits = (sum_i v_[b,i] / 512) * q_  -> batchnorm -> fc -> out [B, 1024]

Sharding (8 cores):
    - att_maps (604 MB, the DMA-bound bulk): batch-sharded, 8 rows/core.
    - activations: every core computes all 64 batches (cheap) from inputs
      rolled so its local 8 batches sit in columns 0..7.
    - fc: output-column sharded (128 cols/core) to split the fc_W read.

Device layout notes:
    - weights are host-transposed so the contraction dim lands on SBUF
      partitions (vWT [512,1536], qWT [768,1536], fcWT [1536,128/core]).
    - v_/q_ are computed transposed: vTa/qTa [128 hk-part, 64 batch-free].
    - per local batch lb: q_ row is assembled at partition 0 (q_rows relu'd
      [8,hk] -> 6KB SBUF->SBUF DMA to [1,hk]), broadcast to 128 partitions
      with a K=1 ones-matmul, then att tiles [128 i, 1536 j] are produced by
      DVE tensor_scalar (per-partition scalar = v_ column) and DMA'd out as
      contiguous 786KB blocks.
    - weight-norm scalars are computed on device (ACT square w/ accum_out,
      ones-matmul partition reduce, sqrt, reciprocal).
"""

import numpy as np

B = 64
NCORES = 8
BL = B // NCORES          # 8 local batches per core
VD, QD, HK, FC = 512, 768, 1536, 1024
OSH = FC // NCORES        # 128 fc output columns per core
MCH = HK // 128           # 12 hk chunks
VKC = VD // 128           # 4
QKC = QD // 128           # 6
NJC = HK // 512           # 3 free-dim chunks of 512
H_DIM = 512.0
BN_EPS = 1e-5

_CACHE = {}


def _kernel_body(ctx, tc, nc, I, att, fco):
    import concourse.bass as bass
    from concourse import mybir

    f32 = mybir.dt.float32
    AF = mybir.ActivationFunctionType
    ALU = mybir.AluOpType

    w = ctx.enter_context(tc.tile_pool(name="w", bufs=1))
    bcp = ctx.enter_context(tc.tile_pool(name="bc", bufs=3))
    ap = ctx.enter_context(tc.tile_pool(name="attp", bufs=6))
    qfp = ctx.enter_context(tc.tile_pool(name="qf", bufs=3))
    scr = ctx.enter_context(tc.tile_pool(name="scr", bufs=2))
    lgp = ctx.enter_context(tc.tile_pool(name="lg", bufs=2))
    # PSUM: every tile slot rounds up to one of the 8 banks; keep the tag
    # budget at 2+1+2+1 = 6 banks.
    psa = ctx.enter_context(tc.tile_pool(name="psa", bufs=2, space="PSUM"))
    psq = ctx.enter_context(tc.tile_pool(name="psq", bufs=1, space="PSUM"))
    psb = ctx.enter_context(tc.tile_pool(name="psb", bufs=2, space="PSUM"))
    pst = ctx.enter_context(tc.tile_pool(name="pst", bufs=1, space="PSUM"))

    # ---- persistent SBUF tiles ----
    vWT = w.tile([128, VKC * HK], f32, tag="vWT")
    qWT = w.tile([128, QKC * HK], f32, tag="qWT")
    fcW = w.tile([128, MCH * OSH], f32, tag="fcW")
    vTi = w.tile([128, VKC * B], f32, tag="vTi")
    qTi = w.tile([128, QKC * B], f32, tag="qTi")
    vTa = w.tile([128, MCH * B], f32, tag="vTa")
    qTa = w.tile([128, MCH * B], f32, tag="qTa")
    logT = w.tile([128, MCH * B], f32, tag="logT")
    qrows = w.tile([BL, HK], f32, tag="qrows")
    qb_row = w.tile([1, HK], f32, tag="qb_row")
    qbs = w.tile([1, HK], f32, tag="qbs")
    vb_c = w.tile([128, MCH], f32, tag="vb_c")
    qb_c = w.tile([128, MCH], f32, tag="qb_c")
    bng = w.tile([128, MCH], f32, tag="bng")
    bnb = w.tile([128, MCH], f32, tag="bnb")
    bnm = w.tile([128, MCH], f32, tag="bnm")
    bnv = w.tile([128, MCH], f32, tag="bnv")
    A_t = w.tile([128, MCH], f32, tag="A_t")
    A5_t = w.tile([128, MCH], f32, tag="A5_t")
    C_t = w.tile([128, MCH], f32, tag="C_t")
    Cm_t = w.tile([128, MCH], f32, tag="Cm_t")
    rsq_t = w.tile([128, MCH], f32, tag="rsq_t")
    fcb = w.tile([B, OSH], f32, tag="fcb")
    ones_r = w.tile([1, 128], f32, tag="ones_r")
    ones_c = w.tile([128, 1], f32, tag="ones_c")
    accv = w.tile([128, VKC], f32, tag="accv")
    accq = w.tile([128, QKC], f32, tag="accq")
    accv1 = w.tile([128, 1], f32, tag="accv1")
    accq1 = w.tile([128, 1], f32, tag="accq1")
    # scalars (each [1,1])
    vg = w.tile([1, 1], f32, tag="vg")
    qg = w.tile([1, 1], f32, tag="qg")
    nv = w.tile([1, 1], f32, tag="nv")
    nq = w.tile([1, 1], f32, tag="nq")
    rnv = w.tile([1, 1], f32, tag="rnv")
    rnq = w.tile([1, 1], f32, tag="rnq")
    rqg = w.tile([1, 1], f32, tag="rqg")
    sv = w.tile([1, 1], f32, tag="sv")
    sq = w.tile([1, 1], f32, tag="sq")
    sqi = w.tile([1, 1], f32, tag="sqi")
    svbc = w.tile([128, 1], f32, tag="svbc")
    sqbc = w.tile([128, 1], f32, tag="sqbc")
    prow = w.tile([1, B], f32, tag="prow")
    pbc = w.tile([128, B], f32, tag="pbc")
    eps_t = w.tile([128, 1], f32, tag="eps_t")

    # ---- input DMAs (emission order ~ priority: norm-critical first) ----
    for kc in range(VKC):
        nc.sync.dma_start(vWT[:, kc * HK:(kc + 1) * HK],
                          I["vWT"][kc * 128:(kc + 1) * 128, :])
    for kc in range(QKC):
        nc.sync.dma_start(qWT[:, kc * HK:(kc + 1) * HK],
                          I["qWT"][kc * 128:(kc + 1) * 128, :])
    for kc in range(VKC):
        nc.sync.dma_start(vTi[:, kc * B:(kc + 1) * B],
                          I["vT"][kc * 128:(kc + 1) * 128, :])
    for kc in range(QKC):
        nc.sync.dma_start(qTi[:, kc * B:(kc + 1) * B],
                          I["qT"][kc * 128:(kc + 1) * 128, :])
    nc.gpsimd.dma_start(vb_c[:], I["vb_c"][:])
    nc.gpsimd.dma_start(qb_c[:], I["qb_c"][:])
    nc.gpsimd.dma_start(qb_row[:], I["qb_row"][:])
    nc.gpsimd.dma_start(vg[:], I["vg"][:])
    nc.gpsimd.dma_start(qg[:], I["qg"][:])
    nc.gpsimd.dma_start(bng[:], I["bng"][:])
    nc.gpsimd.dma_start(bnb[:], I["bnb"][:])
    nc.gpsimd.dma_start(bnm[:], I["bnm"][:])
    nc.gpsimd.dma_start(bnv[:], I["bnv"][:])
    nc.gpsimd.dma_start(fcb[:], I["fcb_rep"][:])
    for jc in range(MCH):
        nc.sync.dma_start(fcW[:, jc * OSH:(jc + 1) * OSH],
                          I["fcWT"][jc * 128:(jc + 1) * 128, :])

    nc.vector.memset(ones_r[:], 1.0)
    nc.vector.memset(ones_c[:], 1.0)
    nc.vector.memset(eps_t[:], BN_EPS)

    # ---- weight norms: ||W||^2 via ACT square + free-dim accumulate, then
    # partition reduce with a ones matmul ----
    for kc in range(VKC):
        t = scr.tile([128, HK], f32, tag="sqscr")
        nc.scalar.activation(t[:], vWT[:, kc * HK:(kc + 1) * HK], AF.Square,
                             accum_out=accv[:, kc:kc + 1])
    for kc in range(QKC):
        t = scr.tile([128, HK], f32, tag="sqscr")
        nc.scalar.activation(t[:], qWT[:, kc * HK:(kc + 1) * HK], AF.Square,
                             accum_out=accq[:, kc:kc + 1])
    nc.vector.tensor_reduce(accv1[:], accv[:], mybir.AxisListType.X, ALU.add)
    nc.vector.tensor_reduce(accq1[:], accq[:], mybir.AxisListType.X, ALU.add)
    nps = pst.tile([1, 1], f32, tag="tiny")
    nc.tensor.matmul(nps[:], accv1[:], ones_c[:], start=True, stop=True)
    nc.scalar.activation(nv[:], nps[:], AF.Sqrt)
    nps2 = pst.tile([1, 1], f32, tag="tiny")
    nc.tensor.matmul(nps2[:], accq1[:], ones_c[:], start=True, stop=True)
    nc.scalar.activation(nq[:], nps2[:], AF.Sqrt)
    nc.vector.reciprocal(rnv[:], nv[:])
    nc.vector.reciprocal(rnq[:], nq[:])
    nc.vector.tensor_tensor(sv[:], rnv[:], vg[:], ALU.mult)
    nc.vector.tensor_tensor(sq[:], rnq[:], qg[:], ALU.mult)
    nc.vector.reciprocal(rqg[:], qg[:])
    nc.vector.tensor_tensor(sqi[:], nq[:], rqg[:], ALU.mult)
    # broadcast s_v, s_q to [128,1] for per-partition ACT scale
    bps = pst.tile([128, 1], f32, tag="tiny")
    nc.tensor.matmul(bps[:], ones_r[:], sv[:], start=True, stop=True)
    nc.scalar.copy(svbc[:], bps[:])
    bps2 = pst.tile([128, 1], f32, tag="tiny")
    nc.tensor.matmul(bps2[:], ones_r[:], sq[:], start=True, stop=True)
    nc.scalar.copy(sqbc[:], bps2[:])

    # ---- activations, transposed: vTa/qTa [128 hk, 64 batch] ----
    for mc in range(MCH):
        ps = psa.tile([128, B], f32, tag="aps")
        for kc in range(VKC):
            nc.tensor.matmul(ps[:],
                             vWT[:, kc * HK + mc * 128: kc * HK + (mc + 1) * 128],
                             vTi[:, kc * B:(kc + 1) * B],
                             start=(kc == 0), stop=(kc == VKC - 1))
        nc.scalar.activation(vTa[:, mc * B:(mc + 1) * B], ps[:], AF.Relu,
                             bias=vb_c[:, mc:mc + 1], scale=svbc[:, 0:1])
    for mc in range(MCH):
        ps = psa.tile([128, B], f32, tag="aps")
        for kc in range(QKC):
            nc.tensor.matmul(ps[:],
                             qWT[:, kc * HK + mc * 128: kc * HK + (mc + 1) * 128],
                             qTi[:, kc * B:(kc + 1) * B],
                             start=(kc == 0), stop=(kc == QKC - 1))
        nc.scalar.activation(qTa[:, mc * B:(mc + 1) * B], ps[:], AF.Relu,
                             bias=qb_c[:, mc:mc + 1], scale=sqbc[:, 0:1])

    # ---- q_ rows for the local 8 batches, batch on partitions ----
    # bias folded into the contraction as an extra K=1 matmul with rhs
    # q_b/s_q, so relu(s*(x@W.T) + b) = relu(s*psum).
    nc.vector.tensor_scalar(qbs[:], qb_row[:], sqi[0:1, 0:1], None, ALU.mult)
    for jc in range(NJC):
        ps = psq.tile([BL, 512], f32, tag="qrp")
        for kc in range(QKC):
            nc.tensor.matmul(ps[:],
                             qTi[:, kc * B: kc * B + BL],
                             qWT[:, kc * HK + jc * 512: kc * HK + (jc + 1) * 512],
                             start=(kc == 0), stop=False)
        nc.tensor.matmul(ps[:], ones_r[0:1, 0:BL],
                         qbs[0:1, jc * 512:(jc + 1) * 512],
                         start=False, stop=True)
        nc.scalar.activation(qrows[:, jc * 512:(jc + 1) * 512], ps[:], AF.Relu,
                             scale=sqbc[0:BL, 0:1])

    # ---- att_maps: per local batch, broadcast q_ row to 128 partitions,
    # then 12 DVE tensor_scalar tiles [128,1536] -> DMA out ----
    for lb in range(BL):
        qf = qfp.tile([1, HK], f32, tag="qf")
        nc.sync.dma_start(qf[0:1, :], qrows[lb:lb + 1, :])
        qbc = bcp.tile([128, HK], f32, tag="qbc")
        for jc in range(NJC):
            ps = psb.tile([128, 512], f32, tag="bcp")
            nc.tensor.matmul(ps[:], ones_r[:],
                             qf[0:1, jc * 512:(jc + 1) * 512],
                             start=True, stop=True)
            nc.scalar.copy(qbc[:, jc * 512:(jc + 1) * 512], ps[:])
        for ic in range(MCH):
            at = ap.tile([128, HK], f32, tag="at")
            nc.vector.tensor_scalar(at[:], qbc[:],
                                    vTa[:, ic * B + lb: ic * B + lb + 1],
                                    None, ALU.mult)
            nc.sync.dma_start(att[lb, ic * 128:(ic + 1) * 128, :], at[:])

    # ---- pooled / batchnorm / fc (off the critical path; emitted last) ----
    pps = pst.tile([1, B], f32, tag="tiny")
    for mc in range(MCH):
        nc.tensor.matmul(pps[:], ones_c[:], vTa[:, mc * B:(mc + 1) * B],
                         start=(mc == 0), stop=(mc == MCH - 1))
    nc.scalar.copy(prow[:], pps[:])
    pbps = pst.tile([128, B], f32, tag="tiny")
    nc.tensor.matmul(pbps[:], ones_r[:], prow[:], start=True, stop=True)
    nc.scalar.copy(pbc[:], pbps[:])
    # bn affine: A = gamma * rsqrt(var+eps); C = beta - mean*A; A5 = A/512
    nc.scalar.activation(rsq_t[:], bnv[:], AF.Sqrt, bias=eps_t[:, 0:1])
    nc.vector.reciprocal(rsq_t[:], rsq_t[:])
    nc.vector.tensor_tensor(A_t[:], bng[:], rsq_t[:], ALU.mult)
    nc.vector.tensor_tensor(Cm_t[:], bnm[:], A_t[:], ALU.mult)
    nc.vector.tensor_tensor(C_t[:], bnb[:], Cm_t[:], ALU.subtract)
    nc.vector.tensor_scalar(A5_t[:], A_t[:], 1.0 / H_DIM, None, ALU.mult)
    for jc in range(MCH):
        t1 = lgp.tile([128, B], f32, tag="l1")
        nc.vector.tensor_scalar(t1[:], qTa[:, jc * B:(jc + 1) * B],
                                A5_t[:, jc:jc + 1], None, ALU.mult)
        t2 = lgp.tile([128, B], f32, tag="l2")
        nc.vector.tensor_tensor(t2[:], t1[:], pbc[:], ALU.mult)
        nc.vector.tensor_scalar(logT[:, jc * B:(jc + 1) * B], t2[:],
                                C_t[:, jc:jc + 1], None, ALU.add)
    fps = pst.tile([B, OSH], f32, tag="tiny")
    for jc in range(MCH):
        nc.tensor.matmul(fps[:], logT[:, jc * B:(jc + 1) * B],
                         fcW[:, jc * OSH:(jc + 1) * OSH],
                         start=(jc == 0), stop=(jc == MCH - 1))
    fcs = lgp.tile([B, OSH], f32, tag="fcs")
    nc.vector.tensor_tensor(fcs[:], fps[:], fcb[:], ALU.add)
    nc.sync.dma_start(fco[:], fcs[:])


def build():
    """Build + compile the Bass module (cached)."""
    if "nc" in _CACHE:
        return _CACHE["nc"]
    from contextlib import ExitStack
    import concourse.tile as tile
    from concourse import bacc, mybir

    f32 = mybir.dt.float32
    nc = bacc.Bacc("TRN2", target_bir_lowering=False, debug=False,
                   enable_asserts=True, num_devices=NCORES)
    I = {}

    def din(name, shape):
        I[name] = nc.dram_tensor(name, list(shape), f32, kind="ExternalInput").ap()

    din("vT", (VD, B))
    din("qT", (QD, B))
    din("vWT", (VD, HK))
    din("qWT", (QD, HK))
    din("fcWT", (HK, OSH))
    din("vb_c", (128, MCH))
    din("qb_c", (128, MCH))
    din("qb_row", (1, HK))
    din("vg", (1, 1))
    din("qg", (1, 1))
    din("bng", (128, MCH))
    din("bnb", (128, MCH))
    din("bnm", (128, MCH))
    din("bnv", (128, MCH))
    din("fcb_rep", (B, OSH))
    att = nc.dram_tensor("att", [BL, HK, HK], f32, kind="ExternalOutput").ap()
    fco = nc.dram_tensor("fco", [B, OSH], f32, kind="ExternalOutput").ap()

    with tile.TileContext(nc) as tc:
        with ExitStack() as ctx:
            _kernel_body(ctx, tc, nc, I, att, fco)
    nc.compile()
    _CACHE["nc"] = nc
    return nc


def make_in_maps(inputs):
    """Host-side prep: transpose/reshape params, roll batches per core."""
    f = lambda x: np.ascontiguousarray(np.asarray(x), dtype=np.float32)
    v, q = f(inputs["v"]), f(inputs["q"])
    vWT = f(np.asarray(inputs["v_W"]).T)
    qWT = f(np.asarray(inputs["q_W"]).T)
    fcWT = f(np.asarray(inputs["fc_W"]).T)
    vb_c = f(np.asarray(inputs["v_b"]).reshape(MCH, 128).T)
    qb_c = f(np.asarray(inputs["q_b"]).reshape(MCH, 128).T)
    qb_row = f(np.asarray(inputs["q_b"]).reshape(1, HK))
    vg = f(np.asarray(inputs["v_g"]).reshape(1, 1))
    qg = f(np.asarray(inputs["q_g"]).reshape(1, 1))
    bng = f(np.asarray(inputs["bn_gamma"]).reshape(MCH, 128).T)
    bnb = f(np.asarray(inputs["bn_beta"]).reshape(MCH, 128).T)
    bnm = f(np.asarray(inputs["bn_mean"]).reshape(MCH, 128).T)
    bnv = f(np.asarray(inputs["bn_var"]).reshape(MCH, 128).T)
    fc_b = np.asarray(inputs["fc_b"])

    in_maps = []
    for c in range(NCORES):
        in_maps.append({
            "vT": f(np.roll(v, -BL * c, axis=0).T),
            "qT": f(np.roll(q, -BL * c, axis=0).T),
            "vWT": vWT,
            "qWT": qWT,
            "fcWT": f(fcWT[:, c * OSH:(c + 1) * OSH]),
            "vb_c": vb_c,
            "qb_c": qb_c,
            "qb_row": qb_row,
            "vg": vg,
            "qg": qg,
            "bng": bng,
            "bnb": bnb,
            "bnm": bnm,
            "bnv": bnv,
            "fcb_rep": f(np.tile(fc_b[c * OSH:(c + 1) * OSH], (B, 1))),
        })
    return in_maps


def assemble(results):
    att_full = np.concatenate([r["att"] for r in results], axis=0)
    out_full = np.empty((B, FC), dtype=np.float32)
    for c in range(NCORES):
        out_full[:, c * OSH:(c + 1) * OSH] = np.roll(results[c]["fco"], BL * c, axis=0)
    return out_full, att_full


def _install_ntff_hook():
    """The agent image lacks antenv.axon_hooks; synthesize it so
    run_bass_kernel_spmd(trace=True) can reach the libaxon NTFF profiler."""
    import sys
    import types
    if "antenv.axon_hooks" in sys.modules:
        return
    try:
        from trn_agent_boot.trn_boot import _ntff_profile_via_ctypes
        hook = _ntff_profile_via_ctypes("/opt/axon/libaxon_pjrt.so")
    except Exception:
        hook = None
    mod = types.ModuleType("antenv.axon_hooks")
    mod._hook = hook
    mod.set_axon_ntff_profile_hook = lambda h: setattr(mod, "_hook", h)
    mod.get_axon_ntff_profile_hook = lambda: mod._hook
    sys.modules["antenv.axon_hooks"] = mod


def run(inputs, trace=False):
    from concourse import bass_utils
    if trace:
        _install_ntff_hook()
    nc = build()
    in_maps = make_in_maps(inputs)
    res = bass_utils.run_bass_kernel_spmd(
        nc, in_maps, core_ids=list(range(NCORES)), trace=trace)
    out_full, att_full = assemble(res.results)
    return (out_full, att_full), res


def kernel(**inputs):
    (out_full, att_full), _ = run(inputs, trace=False)
    return out_full, att_full
